# revision 27
# baseline (speedup 1.0000x reference)
"""Trainium2 Bass kernel for the torchhd-style MNIST HDC encoder model.

Computation (see reference):
    idx   = clip(round(x.reshape(B, P) * 255), 0, 255)            # [B, P] ints
    bund  = sum_p position[p, :] * level_weight[idx[b, p], :]     # [B, D]
    enc   = where(bund > 0, 1, -1)                                # [B, D]
    logit = enc @ classify_weight.T                               # [B, C]

Strategy: shard the hypervector dimension D=10000 across 8 cores (1250 cols
each, zero-padded to 1280).  Everything on the bind/bundle path is +-1, so
it is carried in fp8 (exact) and the bind multiply degenerates to a sign
flip.  Per core, per batch image:
  - dma_gather pulls the 784 indexed fp8 level rows from HBM into SBUF,
    row r landing on partition r%128, block r//128 (7 blocks; block 6
    holds only rows 784..783 -> partitions 0..15, the rest is kept zero).
  - The bind pos*lvl is a bitwise XOR of the position SIGN bits into the
    gathered fp8 bytes, done on DVE over uint16-viewed data (2x mode).
  - TensorE contracts pixel rows with a batch-selector lhsT in fp8
    DoubleRow mode.  Images are processed in PAIRS: one DR pass per block
    j takes plane0 = img 2t's block j and plane1 = img 2t+1's block j
    (plane stride = one buffer), so 7 passes/pair cover both images and
    the zeroed tail partitions contribute exact zeros.
  - ACT applies sign(x - 0.5); TensorE transposes and runs the classify
    matmul in fp32; each core emits partial logits [10, 64] over its D
    chunk, summed on the host.
"""

import os
import sys

for _p in ("/opt/trn_rl_repo", "/root/.axon_site/_ro/trn_rl_repo"):
    if os.path.isdir(_p) and _p not in sys.path:
        sys.path.insert(0, _p)

import ml_dtypes
import numpy as np

BATCH = 64
P = 784            # 28*28 pixels
D = 10000          # hypervector dim
L = 256            # quantization levels
C = 10             # classes
NCORES = 8
DC = D // NCORES   # 1250 real cols per core
DP = 1280          # padded cols (1280B fp8 rows: dma_gather needs %256B)
GBLK = 6           # full 128-row blocks per image (768 of 784 pixels)
NTG = 8            # tail groups: 16-pixel tails of 8 images = 1 block
PAIRS = BATCH // 2

_compiled = None


def _build_bass():
    import concourse.bacc as bacc
    import concourse.tile as tile
    from concourse import mybir

    fp32 = mybir.dt.float32
    bf16 = mybir.dt.bfloat16
    fp8 = mybir.dt.float8e4
    u16 = mybir.dt.uint16
    i16 = mybir.dt.int16

    nc = bacc.Bacc("TRN2", target_bir_lowering=False, debug=False,
                   enable_asserts=False, num_swdge_queues=4)

    lvl = nc.dram_tensor("lvl", [L, DP], fp8, kind="ExternalInput")
    posx = nc.dram_tensor("posx", [128, GBLK * DP // 2], u16,
                          kind="ExternalInput")
    selw = nc.dram_tensor("selw", [128, PAIRS * 2 * BATCH], fp8,
                          kind="ExternalInput")
    clsw = nc.dram_tensor("clsw", [128, (DP // 128) * C], fp32,
                          kind="ExternalInput")
    idxw = nc.dram_tensor("idxw", [128, BATCH * (P // 16)], i16,
                          kind="ExternalInput")
    posxt = nc.dram_tensor("posxt", [128, DP // 2], u16,
                           kind="ExternalInput")
    idxtw = nc.dram_tensor("idxtw", [128, NTG * 8], i16,
                           kind="ExternalInput")
    tselw = nc.dram_tensor("tselw", [128, (NTG // 2) * 2 * BATCH], fp8,
                           kind="ExternalInput")
    identw = nc.dram_tensor("identw", [BATCH, BATCH], bf16,
                            kind="ExternalInput")
    biasw = nc.dram_tensor("biasw", [BATCH, 1], fp32, kind="ExternalInput")
    out = nc.dram_tensor("logitT", [C, BATCH], fp32, kind="ExternalOutput")

    NIDX = P // 16        # 49 idx columns per image
    KT = DP // 128        # 10 classify contraction tiles
    CHUNKS = [(0, 512), (512, 512), (1024, DP - 1024)]  # psum-bank chunks
    BW = GBLK * DP        # bytes per image buffer per partition

    with tile.TileContext(nc) as tc:
        with (
            tc.tile_pool(name="const", bufs=1) as cpool,
            tc.tile_pool(name="gath", bufs=1) as gpool,
            tc.tile_pool(name="prod", bufs=1) as ppool,
            tc.tile_pool(name="misc", bufs=1) as mpool,
            tc.tile_pool(name="psum", bufs=1, space="PSUM") as psum,
            tc.tile_pool(name="psumt", bufs=3, space="PSUM") as psumt,
        ):
            # the FIRST Pool-engine instruction must be a gather so the Q7
            # gather-library load + queue warm-up starts immediately; its
            # memset dependencies run on the (idle) DVE instead
            idx_dummy = cpool.tile([128, 8], i16)
            warm_sb = cpool.tile([128, 2 * 512], fp8)
            nc.vector.memset(idx_dummy[:], 0)
            nc.vector.memset(warm_sb[:], 0)
            g_dummy = cpool.tile([128, 4 * DP], fp8)
            for q in range(4):
                nc.gpsimd.dma_gather(
                    g_dummy[:, q * DP:(q + 1) * DP].rearrange(
                        "p (n m) -> p n m", m=DP),
                    lvl.ap(), idx_dummy[:],
                    num_idxs=128, num_idxs_reg=128, elem_size=DP,
                    queue_num=q,
                )

            # HAM warm-up on the zeroed scratch tile: ~7us of dummy matmuls
            # so the PE clock is at full rate when the real stream starts
            # (output is a scratch bank, never read)
            warm_ps = psum.tile([BATCH, 512], fp32)
            warm_lhsT = warm_sb[:, :2 * BATCH].rearrange(
                "p (t m) -> p t m", t=2)
            warm_rhs = warm_sb[:].rearrange("p (t m) -> p t m", t=2)
            for w in range(22):
                nc.tensor.matmul(
                    warm_ps[:], warm_lhsT, warm_rhs,
                    start=(w == 0), stop=(w == 21),
                    perf_mode=mybir.MatmulPerfMode.DoubleRow,
                )

            # split the index load: a small head covering the first few
            # images lands early so gathers start before the bulk arrives
            IDXHEAD = 8 * NIDX
            idx_sb = cpool.tile([128, BATCH * NIDX], i16)
            nc.sync.dma_start(idx_sb[:, :IDXHEAD], idxw.ap()[:, :IDXHEAD])

            posx_sb = cpool.tile([128, GBLK * DP // 2], u16)
            sel_sb = cpool.tile([128, PAIRS * 2 * BATCH], fp8)
            cls_sb = cpool.tile([128, KT * C], fp32)
            id_sb = cpool.tile([BATCH, BATCH], bf16)
            bias_t = cpool.tile([BATCH, 1], fp32)

            bund = psum.tile([BATCH, DP], fp32)

            posxt_sb = cpool.tile([128, DP // 2], u16)
            idxt_sb = cpool.tile([128, NTG * 8], i16)
            tsel_sb = cpool.tile([128, (NTG // 2) * 2 * BATCH], fp8)
            gtail = cpool.tile([128, NTG * DP], fp8)
            prtail = cpool.tile([128, NTG * DP], fp8)

            NGBUF = 11
            NPBUF = 8
            gbig = gpool.tile([128, NGBUF * BW], fp8)
            prbig = ppool.tile([128, NPBUF * BW], fp8)
            g_tiles = [gbig[:, i * BW:(i + 1) * BW] for i in range(NGBUF)]
            pr_tiles = [prbig[:, i * BW:(i + 1) * BW] for i in range(NPBUF)]

            pr4 = prbig[:].rearrange("p (i b m) -> p i b m", b=GBLK, m=DP)
            sel4 = sel_sb[:].rearrange("p (t o c) -> p t o c", o=2, c=BATCH)

            def gather_xor(b):
                g = g_tiles[b % NGBUF]
                pr = pr_tiles[b % NPBUF]
                g3 = g[:].rearrange("p (n m) -> p n m", m=DP)
                nc.gpsimd.dma_gather(
                    g3[:, :GBLK, :], lvl.ap(),
                    idx_sb[:, b * NIDX:b * NIDX + 8 * GBLK],
                    num_idxs=GBLK * 128, num_idxs_reg=GBLK * 128,
                    elem_size=DP, queue_num=b % 4,
                )
                # bind: pos * lvl for +-1 values == XOR of position sign
                # bits (contiguous 3840-word u16 AP -> DVE 2x mode)
                nc.vector.tensor_tensor(
                    pr[:].bitcast(u16), g[:].bitcast(u16), posx_sb[:],
                    op=mybir.AluOpType.bitwise_xor,
                )

            HS = 3  # split point (blocks) for the last pair's gathers

            def gather_xor_split(b):
                g = g_tiles[b % NGBUF]
                pr = pr_tiles[b % NPBUF]
                g3 = g[:].rearrange("p (n m) -> p n m", m=DP)
                nc.gpsimd.dma_gather(
                    g3[:, :HS, :], lvl.ap(),
                    idx_sb[:, b * NIDX:b * NIDX + 8 * HS],
                    num_idxs=HS * 128, num_idxs_reg=HS * 128, elem_size=DP,
                    queue_num=b % 4,
                )
                nc.vector.tensor_tensor(
                    pr[:, :HS * DP].bitcast(u16),
                    g[:, :HS * DP].bitcast(u16),
                    posx_sb[:, :HS * DP // 2],
                    op=mybir.AluOpType.bitwise_xor,
                )
                nc.gpsimd.dma_gather(
                    g3[:, HS:GBLK, :], lvl.ap(),
                    idx_sb[:, b * NIDX + 8 * HS:b * NIDX + 8 * GBLK],
                    num_idxs=(GBLK - HS) * 128,
                    num_idxs_reg=(GBLK - HS) * 128, elem_size=DP,
                    queue_num=b % 4,
                )
                nc.vector.tensor_tensor(
                    pr[:, HS * DP:].bitcast(u16),
                    g[:, HS * DP:].bitcast(u16),
                    posx_sb[:, HS * DP // 2:],
                    op=mybir.AluOpType.bitwise_xor,
                )

            def tail_gather_xor(g_):
                nc.gpsimd.dma_gather(
                    gtail[:, g_ * DP:(g_ + 1) * DP].rearrange(
                        "p (n m) -> p n m", m=DP),
                    lvl.ap(), idxt_sb[:, g_ * 8:(g_ + 1) * 8],
                    num_idxs=128, num_idxs_reg=128, elem_size=DP,
                    queue_num=g_ % 4,
                )
                nc.vector.tensor_tensor(
                    prtail[:, g_ * DP:(g_ + 1) * DP].bitcast(u16),
                    gtail[:, g_ * DP:(g_ + 1) * DP].bitcast(u16),
                    posxt_sb[:],
                    op=mybir.AluOpType.bitwise_xor,
                )

            # posx must be issued before any XOR reads it (program order IS
            # the dependency order); the rest of the constant loads are
            # interleaved with the first images' gathers so the gather
            # stream owns the DMA engines from the start
            nc.sync.dma_start(posx_sb[:], posx.ap())
            nc.sync.dma_start(posxt_sb[:], posxt.ap())
            nc.sync.dma_start(idxt_sb[:], idxtw.ap())
            for g_ in range(NTG):
                tail_gather_xor(g_)
            gather_xor(0)
            gather_xor(1)
            gather_xor(2)
            gather_xor(3)
            nc.sync.dma_start(idx_sb[:, IDXHEAD:], idxw.ap()[:, IDXHEAD:])
            gather_xor(4)
            gather_xor(5)
            nc.sync.dma_start(sel_sb[:], selw.ap())
            nc.sync.dma_start(tsel_sb[:], tselw.ap())
            gather_xor(6)
            gather_xor(7)
            nc.sync.dma_start(cls_sb[:], clsw.ap())
            nc.sync.dma_start(id_sb[:], identw.ap())
            nc.sync.dma_start(bias_t[:], biasw.ap())

            # one DR pass per block j: plane0 = img 2t, plane1 = img 2t+1
            # (plane stride = one pr buffer); selector maps the planes to
            # output columns 2t / 2t+1.  The shared tail blocks (16-pixel
            # tails of 8 images each) are gathered early -- they only need
            # indices -- and their 4 DR passes are spread mid-stream.
            gt4 = gtail[:].rearrange("p (i m) -> p i m", m=DP)
            prt4 = prtail[:].rearrange("p (i m) -> p i m", m=DP)
            tsel4 = tsel_sb[:].rearrange("p (t o c) -> p t o c",
                                         o=2, c=BATCH)
            for t in range(PAIRS - 1):
                for b in (2 * t, 2 * t + 1):
                    if b >= 8:
                        gather_xor(b)
                e = (2 * t) % NPBUF
                for j in range(GBLK):
                    for (c0, cn) in CHUNKS:
                        nc.tensor.matmul(
                            bund[:, c0:c0 + cn],
                            sel4[:, t],
                            pr4[:, e:e + 2, j, c0:c0 + cn],
                            start=(t == 0 and j == 0),
                            stop=False,
                            perf_mode=mybir.MatmulPerfMode.DoubleRow,
                        )
                if 8 <= t < 8 + NTG // 2:
                    tp_ = t - 8
                    for (c0, cn) in CHUNKS:
                        nc.tensor.matmul(
                            bund[:, c0:c0 + cn],
                            tsel4[:, tp_],
                            prt4[:, 2 * tp_:2 * tp_ + 2, c0:c0 + cn],
                            start=False,
                            stop=False,
                            perf_mode=mybir.MatmulPerfMode.DoubleRow,
                        )

            # last pair: split gathers so the XOR halves start earlier;
            # blocks 0..2 matmuls run off the first halves, blocks 3..5 go
            # chunk-major with per-chunk sign + classify interleaved
            t = PAIRS - 1
            gather_xor_split(2 * t)
            gather_xor_split(2 * t + 1)

            e = (2 * t) % NPBUF
            enc = mpool.tile([BATCH, DP], bf16)
            logit_ps = psum.tile([C, BATCH], fp32)

            for j in range(HS):
                for (c0, cn) in CHUNKS:
                    nc.tensor.matmul(
                        bund[:, c0:c0 + cn],
                        sel4[:, t],
                        pr4[:, e:e + 2, j, c0:c0 + cn],
                        start=False,
                        stop=False,
                        perf_mode=mybir.MatmulPerfMode.DoubleRow,
                    )

            def chunk_mms(ci):
                c0, cn = CHUNKS[ci]
                for j in range(HS, GBLK):
                    nc.tensor.matmul(
                        bund[:, c0:c0 + cn],
                        sel4[:, t],
                        pr4[:, e:e + 2, j, c0:c0 + cn],
                        start=False,
                        stop=(j == GBLK - 1),
                        perf_mode=mybir.MatmulPerfMode.DoubleRow,
                    )

            def sign_chunk(ci):
                # integer sums; -0.5 bias makes where(x>0,1,-1) exact
                c0, cn = CHUNKS[ci]
                nc.scalar.activation(enc[:, c0:c0 + cn], bund[:, c0:c0 + cn],
                                     mybir.ActivationFunctionType.Sign,
                                     bias=bias_t[:])

            def classify_kts(k0, k1):
                # transpose 128-col chunks of enc, then fp32 matmul
                for kt in range(k0, k1):
                    tp = psumt.tile([128, BATCH], bf16, name="tp")
                    nc.tensor.transpose(
                        tp[:], enc[:, kt * 128:(kt + 1) * 128], id_sb[:])
                    etc = mpool.tile([128, BATCH], fp32, name="etc", bufs=4)
                    nc.scalar.copy(etc[:], tp[:])
                    nc.tensor.matmul(
                        logit_ps[:], cls_sb[:, kt * C:(kt + 1) * C], etc[:],
                        start=(kt == 0), stop=(kt == KT - 1),
                    )

            chunk_mms(0)
            chunk_mms(1)
            sign_chunk(0)
            classify_kts(0, 4)
            chunk_mms(2)
            sign_chunk(1)
            classify_kts(4, 8)
            sign_chunk(2)
            classify_kts(8, KT)

            logit_sb = mpool.tile([C, BATCH], fp32)
            nc.scalar.copy(logit_sb[:], logit_ps[:])
            nc.sync.dma_start(out.ap(), logit_sb[:])

    nc.compile()
    return nc


def _prep_inputs(x, position, level_weight, classify_weight):
    """Host-side shard prep: returns in_maps for the 8 cores."""
    xf = x.reshape(BATCH, P).astype(np.float32)
    idx = np.clip(np.round(xf * np.float32(L - 1)), 0, L - 1).astype(np.int16)
    # dma_gather wraps indices as [16, n/16]: index j at [j%16, j//16],
    # replicated across all 128 partitions
    idxw = np.ascontiguousarray(
        idx.reshape(BATCH, P // 16, 16).transpose(2, 0, 1)
    ).reshape(16, BATCH * (P // 16))
    idxw = np.tile(idxw, (8, 1))  # [128, ...]

    # tail indices: group g covers images 8g..8g+7, pixels 768..783; index
    # j = 16*i + k (image i, tail pixel k) lands at [j%16=k, j//16=i]
    idxt = np.concatenate(
        [idx[8 * g:8 * g + 8, GBLK * 128:].T for g in range(NTG)], axis=1)
    idxtw = np.tile(idxt, (8, 1)).astype(np.int16)  # [128, NTG*8]

    # tail-pass selectors: DR pass p pairs tail blocks 2p (images 16p..
    # 16p+7, row r -> col 16p + r//16) and 2p+1 (images 16p+8..16p+15)
    tsel = np.zeros((128, NTG // 2, 2, BATCH), np.float32)
    for p_ in range(NTG // 2):
        for r in range(128):
            tsel[r, p_, 0, 16 * p_ + r // 16] = 1.0
            tsel[r, p_, 1, 16 * p_ + 8 + r // 16] = 1.0
    tselw = tsel.reshape(128, (NTG // 2) * 2 * BATCH).astype(
        ml_dtypes.float8_e4m3)

    # pair selectors: plane o of pair t -> one-hot output column 2t+o,
    # duplicated across all 128 contraction partitions
    sel = np.zeros((128, PAIRS, 2, BATCH), np.float32)
    for t in range(PAIRS):
        sel[:, t, 0, 2 * t] = 1.0
        sel[:, t, 1, 2 * t + 1] = 1.0
    selw = sel.reshape(128, PAIRS * 2 * BATCH).astype(ml_dtypes.float8_e4m3)

    identw = np.eye(BATCH, dtype=np.float32).astype(ml_dtypes.bfloat16)

    KT = DP // 128
    in_maps = []
    for core in range(NCORES):
        cols = slice(core * DC, (core + 1) * DC)

        lvl = np.zeros((L, DP), ml_dtypes.float8_e4m3)
        lvl[:, :DC] = level_weight[:, cols].astype(ml_dtypes.float8_e4m3)

        # position sign bits, gather-layout [part, blk, d], packed as u16
        pos = np.zeros((GBLK * 128, DP), np.float32)
        pos[:, :DC] = position[:GBLK * 128, cols]
        signs = (pos < 0).astype(np.uint8) << 7
        posx = np.ascontiguousarray(
            signs.reshape(GBLK, 128, DP).transpose(1, 0, 2)
        ).reshape(128, GBLK * DP).view(np.uint16)

        # tail-block position signs: partition r = tail pixel 768 + r%16
        post = np.zeros((128, DP), np.float32)
        post[:, :DC] = position[GBLK * 128 + np.arange(128) % 16][:, cols]
        posxt = ((post < 0).astype(np.uint8) << 7).reshape(
            128, DP).view(np.uint16)

        cls = np.zeros((C, DP), np.float32)
        cls[:, :DC] = classify_weight[:, cols]
        clsw = np.ascontiguousarray(
            cls.reshape(C, KT, 128).transpose(2, 1, 0)
        ).reshape(128, KT * C)

        in_maps.append({
            "lvl": lvl,
            "posx": posx,
            "posxt": posxt,
            "idxtw": idxtw,
            "tselw": tselw,
            "selw": selw,
            "clsw": clsw,
            "idxw": idxw,
            "identw": identw,
            "biasw": np.full((BATCH, 1), -0.5, np.float32),
        })
    return in_maps


def kernel(x, position, level_weight, classify_weight, _run_kwargs=None):
    global _compiled
    if _compiled is None:
        _compiled = _build_bass()
    nc = _compiled

    import concourse.bass_utils as bass_utils

    in_maps = _prep_inputs(x, position, level_weight, classify_weight)
    res = bass_utils.run_bass_kernel_spmd(
        nc, in_maps, core_ids=list(range(NCORES)), **(_run_kwargs or {})
    )
    logit = np.zeros((BATCH, C), np.float32)
    for core in range(NCORES):
        logit += res.results[core]["logitT"].T.astype(np.float32)
    kernel.last_result = res
    return logit
